# revision 19
# baseline (speedup 1.0000x reference)
"""Trainium2 Bass kernel for nn_ColorDecoder (segment_reduce).

Reference computation (per sample):
  logits = conv1x1(feature_map)            [21, 64, 64]
  seg    = softmax_k(logits)
  seg_up = bilinear_upsample(seg, 512, 512)          (never materialized!)
  q      = einsum('chw,khw->kc', x, seg_up) / (H*W)  [21, 3]
  attn   = einsum('chw,kc->khw', x, q)               [21, 512, 512]

Key algebraic trick: bilinear upsampling U is linear, so
  q[k,c] = sum_hw seg[k,hw] * (U_y^T x_c U_x)[hw] / (H*W)
which needs only the 64x64 adjoint-downsampled x — the 512x512 seg_up is
never computed.  The output attn is a rank-3 broadcast computed by a
block-diagonal PE matmul (6 spatial strips x 21 classes packed into 126
PSUM partitions).  The image is host-padded from 512 to 528 rows so all
6 strips are a uniform 88 rows: the flattened (class, strip) index
j = 6k+i then has a uniform 88*W DRAM stride and each store batch is a
single 126-descriptor 2D-AP DMA that spreads across all 16 SDMA engines
(the scratch rows are sliced off on the host).

Output is written fp16 (tolerance 2e-2 >> fp16's ~5e-4) and upcast on
the host — halves store traffic.  All matmul streams are fp16 (the PE
runs clock-pinned at 1.2 GHz here, 1 column/cycle at fp16), with fp32
PSUM accumulation throughout.

The per-sample pre-attn pipeline (logits -> transpose -> exp, bilinear
adjoint downsample, q, W_pack) is software-pipelined: its instructions
are interleaved between the previous sample's attn windows via a step
generator, and the fm/x loads are split ~1MB so the xi window prefetches
never queue behind them in the SWDGE FIFO.

Sharding: pure data parallel, batch 16 -> 2 samples on each of 8 cores.
"""

import numpy as np

import bass_rust
import concourse.bass as bass
import concourse.mybir as mybir
from concourse.ap import AP
from concourse.tile import TileContext, ScopedClock
from concourse.bass_utils import run_bass_kernel_spmd

# ---------------------------------------------------------------------------
# Workaround for this walrus build: instructions carrying more than one
# semaphore wait fail codegen ("Too many sync wait commands").  Hoist excess
# waits onto preceding same-engine InstNoOps; same for the end-of-kernel
# drain.
# ---------------------------------------------------------------------------
_MAX_WAITS = 1
_orig_commit = TileContext._commit_instruction


def _commit_split(self, inst, lazy_reg_writes: bool = True):
    si = getattr(inst, "sync_info", None)
    if si is not None and len(si.on_wait) > _MAX_WAITS:
        waits = list(si.on_wait)
        extra, keep = waits[:-_MAX_WAITS], waits[-_MAX_WAITS:]
        for wt in extra:
            nop = mybir.InstNoOp(
                name=self.nc.get_next_instruction_name(),
                sync_info=mybir.SyncInfo(on_wait=[wt], on_update=[]),
                bass_nofuse=True,
                engine=inst.engine,
            )
            _orig_commit(self, nop, lazy_reg_writes)
        inst.sync_info = mybir.SyncInfo(on_wait=keep, on_update=list(si.on_update))
    return _orig_commit(self, inst, lazy_reg_writes)


def _patched_drain_and_barrier(self, tick_clock, wait_clock):
    drain_inst = self.nc.sync.drain()
    wait_clock.add_sem_waits(
        drain_inst.ins, ScopedClock({None: tick_clock.global_clock})
    )
    si = drain_inst.ins.sync_info
    waits = list(si.on_wait) if si else []
    if len(waits) > _MAX_WAITS:
        drain_inst.ins.sync_info = bass_rust.SyncInfo(on_wait=[], on_update=[])
        by_name = {hh.name: hh for hh in self.sems.allocated().values()}
        for wt in waits:
            self.nc.sync.nop().wait_op(by_name[wt.ant_name], wt.wait_value, "sem-ge")
    self.nc.all_engine_barrier()
    assert self.sems is not None
    popped = self.nc._tile_sem_poison_stack.pop()
    assert popped is self._sem_poison
    self.nc.clear_and_free_semaphores(list(self.sems.allocated().values()))
    self.nc.all_engine_barrier()


TileContext._commit_instruction = _commit_split
TileContext._drain_and_barrier = _patched_drain_and_barrier



# ---------------------------------------------------------------------------
# Problem geometry (hardcoded per spec)
# ---------------------------------------------------------------------------
B, F, SH, SW = 16, 256, 64, 64      # feature map
H, W = 512, 512                     # image
K = 21                              # classes
NCORES = 8
BPC = B // NCORES                   # samples per core = 2
HW = H * W                          # 262144
SHW = SH * SW                       # 4096
NREP = 6                            # spatial strips
STRIP = 88                          # rows per strip (H padded to 528)
HP = NREP * STRIP                   # 528 padded image rows
HWP = HP * W                        # padded image pixels
JP = K * NREP                       # 126 packed output partitions, j = 6k+i
GB = 8                              # rows per XI window == store batch
NW = STRIP // GB                    # 11 windows per sample

F32 = mybir.dt.float32
F16 = mybir.dt.float16
BF16 = mybir.dt.bfloat16


def _upsample_matrix(n_in, n_out):
    """align_corners=True bilinear interpolation matrix [n_out, n_in]."""
    u = np.zeros((n_out, n_in), dtype=np.float64)
    pos = np.linspace(0.0, n_in - 1.0, n_out)
    i0 = np.floor(pos).astype(np.int64)
    i1 = np.minimum(i0 + 1, n_in - 1)
    frac = pos - i0
    np.add.at(u, (np.arange(n_out), i0), 1.0 - frac)
    np.add.at(u, (np.arange(n_out), i1), frac)
    return u.astype(np.float32)


def _host_consts(conv_w, conv_b):
    uy = _upsample_matrix(SH, H)            # [512, 64]
    ux = _upsample_matrix(SW, W)            # [512, 64]
    idn64 = np.eye(64, dtype=np.float16)
    # attn contraction row index is r = 4*c + i  (c channel, i strip);
    # packed output column j = 4*k + i.
    i3r = np.zeros((3, 3 * NREP), dtype=np.float16)
    for i in range(NREP):
        for c in range(3):
            i3r[c, NREP * c + i] = 1.0
    mask = np.zeros((3 * NREP, JP), dtype=np.float16)
    for i in range(NREP):
        for c in range(3):
            for k in range(K):
                mask[NREP * c + i, NREP * k + i] = 1.0
    return {
        "convwT": np.ascontiguousarray(conv_w.T).astype(np.float16),  # [256,21]
        "convb": conv_b.reshape(1, K).astype(np.float16),
        "uy": uy.astype(np.float16),
        "ux": ux.astype(np.float16),
        "idn64": idn64,
        "i3r": i3r,
        "maskblk": mask,
        "ones512": np.ones((1, 512), dtype=np.float16),
    }


def _build(with_bias: bool, loop: int = 1):
    nc = bass.Bass("TRN2", target_bir_lowering=False, debug=False)

    fm_d = nc.dram_tensor("fm", [BPC, F, SHW], F32, kind="ExternalInput").ap()
    x_d = nc.dram_tensor("x", [BPC, 3, HWP], F32, kind="ExternalInput").ap()
    convwT_d = nc.dram_tensor("convwT", [F, K], F16, kind="ExternalInput").ap()
    convb_d = nc.dram_tensor("convb", [1, K], F16, kind="ExternalInput").ap()
    uy_d = nc.dram_tensor("uy", [H, SH], F16, kind="ExternalInput").ap()
    ux_d = nc.dram_tensor("ux", [W, SW], F16, kind="ExternalInput").ap()
    idn_d = nc.dram_tensor("idn64", [64, 64], F16, kind="ExternalInput").ap()
    i3r_d = nc.dram_tensor("i3r", [3, 3 * NREP], F16, kind="ExternalInput").ap()
    mask_d = nc.dram_tensor("maskblk", [3 * NREP, JP], F16,
                            kind="ExternalInput").ap()
    ones_d = nc.dram_tensor("ones512", [1, 512], F16, kind="ExternalInput").ap()
    out_d = nc.dram_tensor("attn", [BPC, K, HWP], F16, kind="ExternalOutput").ap()

    with TileContext(nc) as tc:
        with (
            tc.tile_pool(name="const", bufs=1) as cpool,
            tc.tile_pool(name="fm", bufs=2) as fmpool,
            tc.tile_pool(name="xc", bufs=2) as xcpool,
            tc.tile_pool(name="seg", bufs=2) as segpool,
            tc.tile_pool(name="xi", bufs=5) as xipool,
            tc.tile_pool(name="stg", bufs=4) as stgpool,
            tc.tile_pool(name="small", bufs=2) as smpool,
            tc.tile_pool(name="ps1", bufs=1, space="PSUM") as ps1,
            tc.tile_pool(name="pst", bufs=2, space="PSUM") as pst,
            tc.tile_pool(name="psw", bufs=1, space="PSUM") as psw,
            tc.tile_pool(name="psa", bufs=4, space="PSUM") as psa,
        ):
            # ---- constants (loaded once) ----
            convwT_s = cpool.tile([128, F // 128, K], F16, tag="convwT")
            nc.sync.dma_start(
                out=convwT_s[:], in_=convwT_d.rearrange("(a p) k -> p a k", p=128)
            )
            convb_s = cpool.tile([1, K], F16, tag="convb")
            nc.sync.dma_start(out=convb_s[:], in_=convb_d[:])
            ones_s = cpool.tile([1, 512], F16, tag="ones512")
            nc.sync.dma_start(out=ones_s[:], in_=ones_d[:])
            uy_s = cpool.tile([128, 4, SH], F16, tag="uy")
            nc.sync.dma_start(
                out=uy_s[:], in_=uy_d.rearrange("(p a) k -> p a k", a=4)
            )
            ux_s = cpool.tile([128, 4, SW], F16, tag="ux")
            nc.sync.dma_start(
                out=ux_s[:], in_=ux_d.rearrange("(a p) k -> p a k", p=128)
            )
            idn_s = cpool.tile([64, 64], F16, tag="idn64")
            nc.sync.dma_start(out=idn_s[:], in_=idn_d[:])
            idn_s32 = cpool.tile([64, 64], F32, tag="idn64f32")
            nc.gpsimd.dma_start(out=idn_s32[:], in_=idn_d[:])
            i3r_s = cpool.tile([3, 3 * NREP], F16, tag="i3r")
            nc.sync.dma_start(out=i3r_s[:], in_=i3r_d[:])
            mask_s = cpool.tile([3 * NREP, JP], F16, tag="maskblk")
            nc.sync.dma_start(out=mask_s[:], in_=mask_d[:])

            wpack_of = {}

            def stage_steps(b):
                """Pre-attn pipeline for sample b as a generator.

                Yield points let the caller interleave these instructions
                between attn windows of the previous sample, so the PE/ACT
                round-trips hide under attn matmuls and the SWDGE loads
                never sit in one big block ahead of xi prefetches.
                """
                # ---- loads, split ~1MB so xi prefetches interleave ----
                fm_s = fmpool.tile([128, 2, SHW], F16, tag="fm")

                def load_fm(part):
                    nc.gpsimd.dma_start(
                        out=fm_s[:, :, 1024 * part : 1024 * part + 1024],
                        in_=AP(
                            tensor=fm_d.tensor,
                            offset=b * F * SHW + 1024 * part,
                            ap=[[SHW, 128], [128 * SHW, 2], [1, 1024]],
                        ),
                    )

                xc_s = xcpool.tile([128, 3, 4, W], F16, tag="xc")

                def load_xc(c):
                    nc.gpsimd.dma_start(
                        out=xc_s[:, c],
                        in_=AP(
                            tensor=x_d.tensor,
                            offset=b * 3 * HWP + c * HWP,
                            ap=[[4 * W, 128], [1, 4 * W]],
                        ),
                    )

                # ---- stage 1 chunk: logits -> transpose -> exp ----
                e2_s = segpool.tile([128, 32 * K], F16, tag="e2")

                def stage1(ch):
                    lg = ps1.tile([21, 512], F32, tag="lg")
                    for kc in range(2):
                        nc.tensor.matmul(
                            lg[:],
                            convwT_s[:, kc, :],
                            fm_s[:, kc, 512 * ch : 512 * ch + 512],
                            start=(kc == 0),
                            stop=(kc == 1) and not with_bias,
                        )
                    if with_bias:
                        nc.tensor.matmul(
                            lg[:], convb_s[:], ones_s[:], start=False, stop=True
                        )
                    lgs = smpool.tile([21, 512], F32, tag="lgs")
                    nc.vector.tensor_copy(lgs[:], lg[:])
                    lgT = pst.tile([128, 4 * K], F32, tag="lgT")
                    for t in range(4):
                        nc.tensor.transpose(
                            lgT[:, K * t : K * t + K],
                            lgs[:, 128 * t : 128 * t + 128],
                            idn_s32[0:21, 0:21],
                        )
                    nc.scalar.activation(
                        e2_s[:, 4 * K * ch : 4 * K * ch + 4 * K],
                        lgT[:],
                        mybir.ActivationFunctionType.Exp,
                    )

                # ---- downsample channel: xs = U_y^T x U_x ----
                xsn32 = smpool.tile([128, 96], F32, tag="xsn32")

                def downsample(c):
                    tp = psw.tile([64, W], F32, tag="w")
                    for q in range(4):
                        nc.tensor.matmul(
                            tp[:],
                            uy_s[:, q, :],
                            xc_s[:, c, q, :],
                            start=(q == 0),
                            stop=(q == 3),
                        )
                    t_s = smpool.tile([64, W], F16, tag="tsb")
                    nc.vector.tensor_copy(t_s[:], tp[:])
                    tT_s = smpool.tile([128, 4 * 64], F16, tag="ttsb")
                    for q in range(4):
                        tTp = psw.tile([128, 64], F16, tag="w")
                        nc.tensor.transpose(
                            tTp[:], t_s[:, 128 * q : 128 * q + 128], idn_s[:]
                        )
                        nc.vector.tensor_copy(
                            tT_s[:, 64 * q : 64 * q + 64], tTp[:]
                        )
                    xsp = psw.tile([128, 32], F32, tag="w")
                    for dlt in range(2):
                        for q in range(4):
                            nc.tensor.matmul(
                                xsp[64 * dlt : 64 * dlt + 64, :],
                                ux_s[:, q, :],
                                tT_s[:, 64 * q + dlt : 64 * q + 64 : 2],
                                start=(q == 0),
                                stop=(q == 3),
                                tile_position=(0, 64 * dlt),
                                skip_group_check=True,
                            )
                    nc.vector.tensor_copy(xsn32[:, 32 * c : 32 * c + 32], xsp[:])

                def fold_q_wpack():
                    s_all = smpool.tile([128, 32], F32, tag="sall")
                    nc.vector.tensor_reduce(
                        s_all[:],
                        e2_s[:].rearrange("p (t k) -> p t k", k=K),
                        axis=mybir.AxisListType.X,
                        op=mybir.AluOpType.add,
                    )
                    r_all = smpool.tile([128, 32], F32, tag="rall")
                    nc.vector.reciprocal(r_all[:], s_all[:])
                    nc.vector.tensor_scalar_mul(r_all[:], r_all[:], 1.0 / HW)
                    xsn16 = smpool.tile([128, 96], F16, tag="xsn16")
                    for c in range(3):
                        nc.vector.tensor_mul(
                            xsn16[:, 32 * c : 32 * c + 32],
                            xsn32[:, 32 * c : 32 * c + 32],
                            r_all[:],
                        )
                    qtp = psw.tile([3, K], F32, tag="w")
                    for pair in range(32):
                        nc.tensor.matmul(
                            qtp[:],
                            xsn16[:, pair : 96 : 32],
                            e2_s[:, K * pair : K * pair + K],
                            start=(pair == 0),
                            stop=(pair == 31),
                        )
                    qt_s = smpool.tile([3, K], F16, tag="qtsb")
                    nc.scalar.copy(qt_s[:], qtp[:])
                    # W_pack [18, 126] = blockdiag(q^T x6), j = 6k+i
                    wrp = psw.tile([3 * NREP, JP], F32, tag="w")
                    nc.tensor.matmul(
                        wrp[:],
                        i3r_s[:],
                        qt_s[:].unsqueeze(2).broadcast_to((3, K, NREP)),
                        start=True,
                        stop=True,
                    )
                    wpack_s = smpool.tile([3 * NREP, JP], F16, tag="wpack")
                    nc.vector.tensor_mul(wpack_s[:], wrp[:], mask_s[:])
                    wpack_of[b] = wpack_s

                steps = [
                    lambda: load_fm(0),
                    lambda: load_fm(1),
                    lambda: stage1(0),
                    lambda: stage1(1),
                    lambda: load_fm(2),
                    lambda: stage1(2),
                    lambda: stage1(3),
                    lambda: load_fm(3),
                    lambda: stage1(4),
                    lambda: stage1(5),
                    lambda: load_xc(0),
                    lambda: stage1(6),
                    lambda: stage1(7),
                    lambda: downsample(0),
                    lambda: load_xc(1),
                    lambda: downsample(1),
                    lambda: load_xc(2),
                    lambda: downsample(2),
                    fold_q_wpack,
                ]
                for step in steps:
                    step()
                    yield

            xi_tiles = {}

            def load_xi(idx, b, wdw):
                g0 = wdw * GB
                # partition r = 6*c + i holds x[b, c, strip-i rows g0..g0+GB)
                xi_s = xipool.tile([3 * NREP, GB * W], F16, tag="xi")
                nc.gpsimd.dma_start(
                    out=xi_s[:],
                    in_=AP(
                        tensor=x_d.tensor,
                        offset=b * 3 * HWP + g0 * W,
                        ap=[[HWP, 3], [STRIP * W, NREP], [1, GB * W]],
                    ),
                )
                xi_tiles[(idx, wdw)] = xi_s

            XI_AHEAD = 4

            def attn_sample(idx, b, feeder, nxt):
                wpack_s = wpack_of[b]
                for wdw in range(NW):
                    ahead = wdw + XI_AHEAD
                    if ahead < NW:
                        load_xi(idx, b, ahead)
                    elif nxt is not None and ahead - NW < XI_AHEAD:
                        load_xi(idx + 1, nxt, ahead - NW)
                    xi_s = xi_tiles.pop((idx, wdw))
                    stg_s = stgpool.tile([JP, GB * W], F16, tag="stg")
                    for g in range(GB):
                        ap_ = psa.tile([JP, 512], F32, tag="attnps")
                        nc.tensor.matmul(
                            ap_[:],
                            wpack_s[:],
                            xi_s[:, g * W : (g + 1) * W],
                            start=True,
                            stop=True,
                        )
                        dst = stg_s[:, g * W : (g + 1) * W]
                        if g in (2, 5, 7):
                            nc.vector.tensor_copy(dst, ap_[:])
                        else:
                            nc.scalar.copy(dst, ap_[:])
                    dst_ap = AP(
                        tensor=out_d.tensor,
                        offset=b * K * HWP + wdw * GB * W,
                        ap=[[STRIP * W, JP], [1, GB * W]],
                    )
                    nc.sync.dma_start(out=dst_ap, in_=stg_s[:])
                    if feeder is not None:
                        nsteps = 3 if wdw < 6 else 1
                        for _ in range(nsteps):
                            next(feeder, None)

            # ---- software-pipelined schedule across samples/iterations ----
            seq = [bb for _ in range(loop) for bb in range(BPC)]
            gen = stage_steps(seq[0])
            for _ in gen:
                pass
            for w in range(XI_AHEAD):
                load_xi(0, seq[0], w)
            for idx, b in enumerate(seq):
                nxt = seq[idx + 1] if idx + 1 < len(seq) else None
                feeder = stage_steps(nxt) if nxt is not None else None
                attn_sample(idx, b, feeder, nxt)
                if feeder is not None:
                    for _ in feeder:
                        pass

    return nc


_cache: dict = {}


def _get_nc(with_bias: bool, loop: int):
    key = (with_bias, loop)
    if key not in _cache:
        _cache[key] = _build(with_bias, loop)
    return _cache[key]


def kernel(feature_map, x, conv_w, conv_b, _loop: int = 1):
    feature_map = np.ascontiguousarray(feature_map, dtype=np.float32)
    x = np.ascontiguousarray(x, dtype=np.float32)
    conv_w = np.ascontiguousarray(conv_w, dtype=np.float32)
    conv_b = np.ascontiguousarray(conv_b, dtype=np.float32)

    with_bias = bool(np.any(conv_b != 0.0))
    nc = _get_nc(with_bias, _loop)
    consts = _host_consts(conv_w, conv_b)

    xpad = np.zeros((B, 3, HP, W), dtype=np.float32)
    xpad[:, :, :H, :] = x.reshape(B, 3, H, W)

    in_maps = []
    for core in range(NCORES):
        b0 = core * BPC
        in_maps.append(
            {
                "fm": feature_map[b0 : b0 + BPC].reshape(BPC, F, SHW),
                "x": xpad[b0 : b0 + BPC].reshape(BPC, 3, HWP),
                **consts,
            }
        )
    res = run_bass_kernel_spmd(nc, in_maps, list(range(NCORES)))
    out = np.concatenate(
        [
            res.results[i]["attn"].reshape(BPC, K, HP, W)[:, :, :H, :]
            for i in range(NCORES)
        ],
        axis=0,
    )
    return out.astype(np.float32)


# revision 21
# speedup vs baseline: 1.0153x; 1.0153x over previous
"""Trainium2 Bass kernel for nn_ColorDecoder (segment_reduce).

Reference computation (per sample):
  logits = conv1x1(feature_map)            [21, 64, 64]
  seg    = softmax_k(logits)
  seg_up = bilinear_upsample(seg, 512, 512)          (never materialized!)
  q      = einsum('chw,khw->kc', x, seg_up) / (H*W)  [21, 3]
  attn   = einsum('chw,kc->khw', x, q)               [21, 512, 512]

Key algebraic trick: bilinear upsampling U is linear, so
  q[k,c] = sum_hw seg[k,hw] * (U_y^T x_c U_x)[hw] / (H*W)
which needs only the 64x64 adjoint-downsampled x — the 512x512 seg_up is
never computed.  The output attn is a rank-3 broadcast computed by a
block-diagonal PE matmul (6 spatial strips x 21 classes packed into 126
PSUM partitions).  The image is host-padded from 512 to 528 rows so all
6 strips are a uniform 88 rows: the flattened (class, strip) index
j = 6k+i then has a uniform 88*W DRAM stride and each store batch is a
single 126-descriptor 2D-AP DMA that spreads across all 16 SDMA engines
(the scratch rows are sliced off on the host).

Output is written fp16 (tolerance 2e-2 >> fp16's ~5e-4) and upcast on
the host — halves store traffic.  All matmul streams are fp16 (the PE
runs clock-pinned at 1.2 GHz here, 1 column/cycle at fp16), with fp32
PSUM accumulation throughout.

The per-sample pre-attn pipeline (logits -> transpose -> exp, bilinear
adjoint downsample, q, W_pack) is software-pipelined: its instructions
are interleaved between the previous sample's attn windows via a step
generator, and the fm/x loads are split ~1MB so the xi window prefetches
never queue behind them in the SWDGE FIFO.

Sharding: pure data parallel, batch 16 -> 2 samples on each of 8 cores.
"""

import numpy as np

import bass_rust
import concourse.bass as bass
import concourse.mybir as mybir
from concourse.ap import AP
from concourse.tile import TileContext, ScopedClock
from concourse.bass_utils import run_bass_kernel_spmd

# ---------------------------------------------------------------------------
# Workaround for this walrus build: instructions carrying more than one
# semaphore wait fail codegen ("Too many sync wait commands").  Hoist excess
# waits onto preceding same-engine InstNoOps; same for the end-of-kernel
# drain.
# ---------------------------------------------------------------------------
_MAX_WAITS = 1
_orig_commit = TileContext._commit_instruction


def _commit_split(self, inst, lazy_reg_writes: bool = True):
    si = getattr(inst, "sync_info", None)
    if si is not None and len(si.on_wait) > _MAX_WAITS:
        waits = list(si.on_wait)
        extra, keep = waits[:-_MAX_WAITS], waits[-_MAX_WAITS:]
        for wt in extra:
            nop = mybir.InstNoOp(
                name=self.nc.get_next_instruction_name(),
                sync_info=mybir.SyncInfo(on_wait=[wt], on_update=[]),
                bass_nofuse=True,
                engine=inst.engine,
            )
            _orig_commit(self, nop, lazy_reg_writes)
        inst.sync_info = mybir.SyncInfo(on_wait=keep, on_update=list(si.on_update))
    return _orig_commit(self, inst, lazy_reg_writes)


def _patched_drain_and_barrier(self, tick_clock, wait_clock):
    drain_inst = self.nc.sync.drain()
    wait_clock.add_sem_waits(
        drain_inst.ins, ScopedClock({None: tick_clock.global_clock})
    )
    si = drain_inst.ins.sync_info
    waits = list(si.on_wait) if si else []
    if len(waits) > _MAX_WAITS:
        drain_inst.ins.sync_info = bass_rust.SyncInfo(on_wait=[], on_update=[])
        by_name = {hh.name: hh for hh in self.sems.allocated().values()}
        for wt in waits:
            self.nc.sync.nop().wait_op(by_name[wt.ant_name], wt.wait_value, "sem-ge")
    self.nc.all_engine_barrier()
    assert self.sems is not None
    popped = self.nc._tile_sem_poison_stack.pop()
    assert popped is self._sem_poison
    self.nc.clear_and_free_semaphores(list(self.sems.allocated().values()))
    self.nc.all_engine_barrier()


TileContext._commit_instruction = _commit_split
TileContext._drain_and_barrier = _patched_drain_and_barrier



# ---------------------------------------------------------------------------
# Problem geometry (hardcoded per spec)
# ---------------------------------------------------------------------------
B, F, SH, SW = 16, 256, 64, 64      # feature map
H, W = 512, 512                     # image
K = 21                              # classes
NCORES = 8
BPC = B // NCORES                   # samples per core = 2
HW = H * W                          # 262144
SHW = SH * SW                       # 4096
NREP = 6                            # spatial strips
STRIP = 88                          # rows per strip (H padded to 528)
HP = NREP * STRIP                   # 528 padded image rows
HWP = HP * W                        # padded image pixels
JP = K * NREP                       # 126 packed output partitions, j = 6k+i
GB = 8                              # rows per XI window == store batch
NW = STRIP // GB                    # 11 windows per sample

F32 = mybir.dt.float32
F16 = mybir.dt.float16
BF16 = mybir.dt.bfloat16


def _upsample_matrix(n_in, n_out):
    """align_corners=True bilinear interpolation matrix [n_out, n_in]."""
    u = np.zeros((n_out, n_in), dtype=np.float64)
    pos = np.linspace(0.0, n_in - 1.0, n_out)
    i0 = np.floor(pos).astype(np.int64)
    i1 = np.minimum(i0 + 1, n_in - 1)
    frac = pos - i0
    np.add.at(u, (np.arange(n_out), i0), 1.0 - frac)
    np.add.at(u, (np.arange(n_out), i1), frac)
    return u.astype(np.float32)


def _host_consts(conv_w, conv_b):
    uy = _upsample_matrix(SH, H)            # [512, 64]
    ux = _upsample_matrix(SW, W)            # [512, 64]
    idn64 = np.eye(64, dtype=np.float16)
    # attn contraction row index is r = 6*c + i  (c channel, i strip);
    # packed output column j = 6*k + i.
    i3r = np.zeros((3, 3 * NREP), dtype=np.float16)
    for i in range(NREP):
        for c in range(3):
            i3r[c, NREP * c + i] = 1.0
    mask = np.zeros((3 * NREP, JP), dtype=np.float16)
    for i in range(NREP):
        for c in range(3):
            for k in range(K):
                mask[NREP * c + i, NREP * k + i] = 1.0
    # q rows land on psum partitions {0, 32, 64}; S65 spreads channel c's
    # q row onto wrp rows r = 6c+i.
    sel65 = np.zeros((65, 3 * NREP), dtype=np.float16)
    for c in range(3):
        for i in range(NREP):
            sel65[32 * c, NREP * c + i] = 1.0
    return {
        "convwT": np.ascontiguousarray(conv_w.T).astype(np.float16),  # [256,21]
        "convb": conv_b.reshape(1, K).astype(np.float16),
        "uy": uy.astype(np.float16),
        "ux": ux.astype(np.float16),
        "idn64": idn64,
        "i3r": i3r,
        "maskblk": mask,
        "ones512": np.ones((1, 512), dtype=np.float16),
        "sel65": sel65,
        "ones128": np.ones((128, 1), dtype=np.float32),
    }


def _build(with_bias: bool, loop: int = 1):
    nc = bass.Bass("TRN2", target_bir_lowering=False, debug=False)

    fm_d = nc.dram_tensor("fm", [BPC, F, SHW], F32, kind="ExternalInput").ap()
    x_d = nc.dram_tensor("x", [BPC, 3, HWP], F32, kind="ExternalInput").ap()
    convwT_d = nc.dram_tensor("convwT", [F, K], F16, kind="ExternalInput").ap()
    convb_d = nc.dram_tensor("convb", [1, K], F16, kind="ExternalInput").ap()
    uy_d = nc.dram_tensor("uy", [H, SH], F16, kind="ExternalInput").ap()
    ux_d = nc.dram_tensor("ux", [W, SW], F16, kind="ExternalInput").ap()
    idn_d = nc.dram_tensor("idn64", [64, 64], F16, kind="ExternalInput").ap()
    i3r_d = nc.dram_tensor("i3r", [3, 3 * NREP], F16, kind="ExternalInput").ap()
    mask_d = nc.dram_tensor("maskblk", [3 * NREP, JP], F16,
                            kind="ExternalInput").ap()
    ones_d = nc.dram_tensor("ones512", [1, 512], F16, kind="ExternalInput").ap()
    sel65_d = nc.dram_tensor("sel65", [65, 3 * NREP], F16, kind="ExternalInput").ap()
    ones128_d = nc.dram_tensor("ones128", [128, 1], F32, kind="ExternalInput").ap()
    out_d = nc.dram_tensor("attn", [BPC, K, HWP], F16, kind="ExternalOutput").ap()

    with TileContext(nc) as tc:
        with (
            tc.tile_pool(name="const", bufs=1) as cpool,
            tc.tile_pool(name="fm", bufs=2) as fmpool,
            tc.tile_pool(name="xc", bufs=2) as xcpool,
            tc.tile_pool(name="seg", bufs=2) as segpool,
            tc.tile_pool(name="xi", bufs=5) as xipool,
            tc.tile_pool(name="stg", bufs=4) as stgpool,
            tc.tile_pool(name="small", bufs=2) as smpool,
            tc.tile_pool(name="ps1", bufs=1, space="PSUM") as ps1,
            tc.tile_pool(name="pst", bufs=2, space="PSUM") as pst,
            tc.tile_pool(name="psw", bufs=1, space="PSUM") as psw,
            tc.tile_pool(name="psa", bufs=4, space="PSUM") as psa,
        ):
            # ---- constants (loaded once) ----
            convwT_s = cpool.tile([128, F // 128, K], F16, tag="convwT")
            nc.sync.dma_start(
                out=convwT_s[:], in_=convwT_d.rearrange("(a p) k -> p a k", p=128)
            )
            convb_s = cpool.tile([1, K], F16, tag="convb")
            nc.sync.dma_start(out=convb_s[:], in_=convb_d[:])
            ones_s = cpool.tile([1, 512], F16, tag="ones512")
            nc.sync.dma_start(out=ones_s[:], in_=ones_d[:])
            uy_s = cpool.tile([128, 4, SH], F16, tag="uy")
            nc.sync.dma_start(
                out=uy_s[:], in_=uy_d.rearrange("(p a) k -> p a k", a=4)
            )
            ux_s = cpool.tile([128, 4, SW], F16, tag="ux")
            nc.sync.dma_start(
                out=ux_s[:], in_=ux_d.rearrange("(a p) k -> p a k", p=128)
            )
            idn_s = cpool.tile([64, 64], F16, tag="idn64")
            nc.sync.dma_start(out=idn_s[:], in_=idn_d[:])
            idn_s32 = cpool.tile([64, 64], F32, tag="idn64f32")
            nc.gpsimd.dma_start(out=idn_s32[:], in_=idn_d[:])
            i3r_s = cpool.tile([3, 3 * NREP], F16, tag="i3r")
            nc.sync.dma_start(out=i3r_s[:], in_=i3r_d[:])
            mask_s = cpool.tile([3 * NREP, JP], F16, tag="maskblk")
            nc.sync.dma_start(out=mask_s[:], in_=mask_d[:])
            sel65_s = cpool.tile([65, 3 * NREP], F16, tag="sel65")
            nc.sync.dma_start(out=sel65_s[:], in_=sel65_d[:])
            ones128_s = cpool.tile([128, 1], F32, tag="ones128")
            nc.sync.dma_start(out=ones128_s[:], in_=ones128_d[:])

            wpack_of = {}

            def stage_steps(b):
                """Pre-attn pipeline for sample b as a generator.

                Yield points let the caller interleave these instructions
                between attn windows of the previous sample, so the PE/ACT
                round-trips hide under attn matmuls and the SWDGE loads
                never sit in one big block ahead of xi prefetches.
                """
                # ---- loads, split ~1MB so xi prefetches interleave ----
                fm_s = fmpool.tile([128, 2, SHW], F16, tag="fm")

                def load_fm(part):
                    nc.gpsimd.dma_start(
                        out=fm_s[:, :, 1024 * part : 1024 * part + 1024],
                        in_=AP(
                            tensor=fm_d.tensor,
                            offset=b * F * SHW + 1024 * part,
                            ap=[[SHW, 128], [128 * SHW, 2], [1, 1024]],
                        ),
                    )

                xc_s = xcpool.tile([128, 3, 4, W], F16, tag="xc")

                def load_xc(c):
                    nc.gpsimd.dma_start(
                        out=xc_s[:, c],
                        in_=AP(
                            tensor=x_d.tensor,
                            offset=b * 3 * HWP + c * HWP,
                            ap=[[4 * W, 128], [1, 4 * W]],
                        ),
                    )

                # ---- stage 1 chunk: logits -> transpose -> exp ----
                e2_s = segpool.tile([128, 32 * K], F16, tag="e2")

                def stage1(ch):
                    lg = ps1.tile([21, 512], F32, tag="lg")
                    for kc in range(2):
                        nc.tensor.matmul(
                            lg[:],
                            convwT_s[:, kc, :],
                            fm_s[:, kc, 512 * ch : 512 * ch + 512],
                            start=(kc == 0),
                            stop=(kc == 1) and not with_bias,
                        )
                    if with_bias:
                        nc.tensor.matmul(
                            lg[:], convb_s[:], ones_s[:], start=False, stop=True
                        )
                    lgs = smpool.tile([21, 512], F32, tag="lgs")
                    nc.vector.tensor_copy(lgs[:], lg[:])
                    lgT = pst.tile([128, 4 * K], F32, tag="lgT")
                    for t in range(4):
                        nc.tensor.transpose(
                            lgT[:, K * t : K * t + K],
                            lgs[:, 128 * t : 128 * t + 128],
                            idn_s32[0:21, 0:21],
                        )
                    nc.scalar.activation(
                        e2_s[:, 4 * K * ch : 4 * K * ch + 4 * K],
                        lgT[:],
                        mybir.ActivationFunctionType.Exp,
                    )

                # ---- downsample channel: xs = U_y^T x U_x ----
                xsn32 = smpool.tile([128, 96], F32, tag="xsn32")

                def downsample(c):
                    tp = psw.tile([64, W], F32, tag="w")
                    for q in range(4):
                        nc.tensor.matmul(
                            tp[:],
                            uy_s[:, q, :],
                            xc_s[:, c, q, :],
                            start=(q == 0),
                            stop=(q == 3),
                        )
                    t_s = smpool.tile([64, W], F16, tag="tsb")
                    nc.vector.tensor_copy(t_s[:], tp[:])
                    tT_s = smpool.tile([128, 4 * 64], F16, tag="ttsb")
                    for q in range(4):
                        tTp = psw.tile([128, 64], F16, tag="w")
                        nc.tensor.transpose(
                            tTp[:], t_s[:, 128 * q : 128 * q + 128], idn_s[:]
                        )
                        nc.vector.tensor_copy(
                            tT_s[:, 64 * q : 64 * q + 64], tTp[:]
                        )
                    xsp = psw.tile([128, 32], F32, tag="w")
                    for dlt in range(2):
                        for q in range(4):
                            nc.tensor.matmul(
                                xsp[64 * dlt : 64 * dlt + 64, :],
                                ux_s[:, q, :],
                                tT_s[:, 64 * q + dlt : 64 * q + 64 : 2],
                                start=(q == 0),
                                stop=(q == 3),
                                tile_position=(0, 64 * dlt),
                                skip_group_check=True,
                            )
                    nc.vector.tensor_copy(xsn32[:, 32 * c : 32 * c + 32], xsp[:])

                def fold_q_wpack():
                    s_all = smpool.tile([128, 32], F32, tag="sall")
                    nc.vector.tensor_reduce(
                        s_all[:],
                        e2_s[:].rearrange("p (t k) -> p t k", k=K),
                        axis=mybir.AxisListType.X,
                        op=mybir.AluOpType.add,
                    )
                    r_all = smpool.tile([128, 32], F32, tag="rall")
                    nc.vector.reciprocal(r_all[:], s_all[:])
                    nc.vector.tensor_scalar_mul(r_all[:], r_all[:], 1.0 / HW)
                    xsn16 = smpool.tile([128, 96], F16, tag="xsn16")
                    for c in range(3):
                        nc.vector.tensor_mul(
                            xsn16[:, 32 * c : 32 * c + 32],
                            xsn32[:, 32 * c : 32 * c + 32],
                            r_all[:],
                        )
                    # q[k,c] = sum_{p,t} e2[p,t,k] * xsn16[p,t,c]: per-c DVE
                    # multiply + reduce over t, then a tiny ones-matmul sums
                    # over partitions into psum rows {0, 32, 64}.
                    qprod = smpool.tile([128, 32 * K], F32, tag="qprod")
                    q3 = smpool.tile([128, 3 * K], F32, tag="q3")
                    qtp65 = psw.tile([65, K], F32, tag="w")
                    for c in range(3):
                        nc.vector.tensor_mul(
                            qprod[:].rearrange("p (t k) -> p t k", k=K),
                            e2_s[:].rearrange("p (t k) -> p t k", k=K),
                            xsn16[:, 32 * c : 32 * c + 32]
                            .unsqueeze(2)
                            .broadcast_to((128, 32, K)),
                        )
                        nc.vector.tensor_reduce(
                            q3[:, K * c : K * c + K],
                            qprod[:].rearrange("p (t k) -> p k t", k=K),
                            axis=mybir.AxisListType.X,
                            op=mybir.AluOpType.add,
                        )
                        nc.tensor.matmul(
                            qtp65[32 * c : 32 * c + 1, :],
                            ones128_s[:],
                            q3[:, K * c : K * c + K],
                            start=True,
                            stop=True,
                            skip_group_check=True,
                        )
                    q65_s = smpool.tile([65, K], F16, tag="q65sb")
                    nc.scalar.copy(q65_s[:], qtp65[:])
                    # W_pack [18, 126] = blockdiag(q^T x6), j = 6k+i
                    wrp = psw.tile([3 * NREP, JP], F32, tag="w")
                    nc.tensor.matmul(
                        wrp[:],
                        sel65_s[:],
                        q65_s[:].unsqueeze(2).broadcast_to((65, K, NREP)),
                        start=True,
                        stop=True,
                    )
                    wpack_s = smpool.tile([3 * NREP, JP], F16, tag="wpack")
                    nc.vector.tensor_mul(wpack_s[:], wrp[:], mask_s[:])
                    wpack_of[b] = wpack_s

                steps = [
                    lambda: load_fm(0),
                    lambda: load_fm(1),
                    lambda: stage1(0),
                    lambda: stage1(1),
                    lambda: load_fm(2),
                    lambda: stage1(2),
                    lambda: stage1(3),
                    lambda: load_fm(3),
                    lambda: stage1(4),
                    lambda: stage1(5),
                    lambda: load_xc(0),
                    lambda: stage1(6),
                    lambda: stage1(7),
                    lambda: downsample(0),
                    lambda: load_xc(1),
                    lambda: downsample(1),
                    lambda: load_xc(2),
                    lambda: downsample(2),
                    fold_q_wpack,
                ]
                for step in steps:
                    step()
                    yield

            xi_tiles = {}

            def load_xi(idx, b, wdw):
                g0 = wdw * GB
                # partition r = 6*c + i holds x[b, c, strip-i rows g0..g0+GB)
                xi_s = xipool.tile([3 * NREP, GB * W], F16, tag="xi")
                nc.gpsimd.dma_start(
                    out=xi_s[:],
                    in_=AP(
                        tensor=x_d.tensor,
                        offset=b * 3 * HWP + g0 * W,
                        ap=[[HWP, 3], [STRIP * W, NREP], [1, GB * W]],
                    ),
                )
                xi_tiles[(idx, wdw)] = xi_s

            XI_AHEAD = 4

            def attn_sample(idx, b, feeder, nxt):
                wpack_s = wpack_of[b]
                for wdw in range(NW):
                    ahead = wdw + XI_AHEAD
                    if ahead < NW:
                        load_xi(idx, b, ahead)
                    elif nxt is not None and ahead - NW < XI_AHEAD:
                        load_xi(idx + 1, nxt, ahead - NW)
                    xi_s = xi_tiles.pop((idx, wdw))
                    stg_s = stgpool.tile([JP, GB * W], F16, tag="stg")
                    for g in range(GB):
                        ap_ = psa.tile([JP, 512], F32, tag="attnps")
                        nc.tensor.matmul(
                            ap_[:],
                            wpack_s[:],
                            xi_s[:, g * W : (g + 1) * W],
                            start=True,
                            stop=True,
                        )
                        dst = stg_s[:, g * W : (g + 1) * W]
                        if g in (2, 5, 7):
                            nc.vector.tensor_copy(dst, ap_[:])
                        else:
                            nc.scalar.copy(dst, ap_[:])
                    dst_ap = AP(
                        tensor=out_d.tensor,
                        offset=b * K * HWP + wdw * GB * W,
                        ap=[[STRIP * W, JP], [1, GB * W]],
                    )
                    nc.sync.dma_start(out=dst_ap, in_=stg_s[:])
                    if feeder is not None:
                        nsteps = 3 if wdw < 6 else 1
                        for _ in range(nsteps):
                            next(feeder, None)

            # ---- software-pipelined schedule across samples/iterations ----
            seq = [bb for _ in range(loop) for bb in range(BPC)]
            gen = stage_steps(seq[0])
            for _ in gen:
                pass
            for w in range(XI_AHEAD):
                load_xi(0, seq[0], w)
            for idx, b in enumerate(seq):
                nxt = seq[idx + 1] if idx + 1 < len(seq) else None
                feeder = stage_steps(nxt) if nxt is not None else None
                attn_sample(idx, b, feeder, nxt)
                if feeder is not None:
                    for _ in feeder:
                        pass

    return nc


_cache: dict = {}


def _get_nc(with_bias: bool, loop: int):
    key = (with_bias, loop)
    if key not in _cache:
        _cache[key] = _build(with_bias, loop)
    return _cache[key]


def kernel(feature_map, x, conv_w, conv_b, _loop: int = 1):
    feature_map = np.ascontiguousarray(feature_map, dtype=np.float32)
    x = np.ascontiguousarray(x, dtype=np.float32)
    conv_w = np.ascontiguousarray(conv_w, dtype=np.float32)
    conv_b = np.ascontiguousarray(conv_b, dtype=np.float32)

    with_bias = bool(np.any(conv_b != 0.0))
    nc = _get_nc(with_bias, _loop)
    consts = _host_consts(conv_w, conv_b)

    xpad = np.zeros((B, 3, HP, W), dtype=np.float32)
    xpad[:, :, :H, :] = x.reshape(B, 3, H, W)

    in_maps = []
    for core in range(NCORES):
        b0 = core * BPC
        in_maps.append(
            {
                "fm": feature_map[b0 : b0 + BPC].reshape(BPC, F, SHW),
                "x": xpad[b0 : b0 + BPC].reshape(BPC, 3, HWP),
                **consts,
            }
        )
    res = run_bass_kernel_spmd(nc, in_maps, list(range(NCORES)))
    out = np.concatenate(
        [
            res.results[i]["attn"].reshape(BPC, K, HP, W)[:, :, :H, :]
            for i in range(NCORES)
        ],
        axis=0,
    )
    return out.astype(np.float32)
